# revision 22
# baseline (speedup 1.0000x reference)
"""Trainium2 Bass kernel for nn_BCTransformer: B=131072 batch of tiny 2-token
4-layer transformer encoder forward passes.

v2: pure data parallel over 8 NeuronCores (16384 batch each).  Feature-major
layout [D=128 partitions, columns=(token, sample)]; super-tiles of 1024
columns (512 samples x 2 tokens); 32 super-tiles per core.

Key structural tricks vs v1:
 - CENTERED RESIDUAL STREAM: the LN centering matrix C = I - J/128 is folded
   host-side into every weight that WRITES the residual stream (out-proj,
   ff2, embedding).  The stream hc is mean-free by construction, so LN
   reduces to rstd = rsqrt(mean(hc^2)+eps) and one multiply - no centering
   matmul, no mean matmul, no eps matmul.
 - fp16 datapath (activations + weights): full PE rate, 2x/4x DVE modes,
   5e-4 rounding.  The variance/rsqrt chain runs in bf16 (wide exponent for
   the bit-trick seed).
 - attention: softmax over S=2 == 0.5 + 0.5*tanh(d/2).  k/v biases fold
   away (k bias cancels in k0-k1; Wout@bv folds into out bias).  q bias is
   accumulated into PSUM by a ones-rhs matmul.  dk/dv/vs are computed by
   PE from yd = y0-y1 / ys = y0+y1 (halves the qkv matmul work), and
   Wout@Wv/2 is prefolded so the (v0+v1)/2 term needs no extra elementwise.
 - rsqrt: bf16 magic-seed (top-16-bit shift trick) + 2 Newton steps in one
   custom DVE op with eps folded into the constants.
 - residual adds ride identity matmuls into PSUM; biases ride the psum->
   sbuf Act copies; LN gains/biases folded into neighbouring weights.
"""
import sys

sys.path.insert(0, "/opt/trn_rl_repo")

import math
from contextlib import ExitStack

import numpy as np
import ml_dtypes

import concourse.bass as bass
import concourse.tile as tile
from concourse import bacc, mybir
from concourse.bass_utils import run_bass_kernel_spmd
from concourse.bass_isa import ReduceOp

# ---------------------------------------------------------------- constants
D = 128
NH = 4
HD = 32
FF = 256
L = 4
S = 2
B = 131072
EPS = 1e-5
NCORES = 8
BP = B // NCORES          # batch per core = 16384
N = 256                   # samples per super-tile
NT = 2 * N                # columns per super-tile (tok0 | tok1)
NTILES = BP // N          # 64
MMC = 512                 # matmul column chunk

F32 = mybir.dt.float32
F32R = mybir.dt.float32r
F16 = mybir.dt.float16
BF16 = mybir.dt.bfloat16
I32 = mybir.dt.int32
I16 = mybir.dt.int16
AF = mybir.ActivationFunctionType
ALU = mybir.AluOpType
BF = ml_dtypes.bfloat16

# ------------------------------------------------- rsqrt bit-trick constants
MAGIC = 0x5F375A86
MAGIC_VH = MAGIC - (1 << 22)       # seed domain: vh = (var+eps)/2
SEED_ADD32 = MAGIC_VH + 1          # seed = ~(i_vh >> 1) + SEED_ADD
SEED_ADD16 = (MAGIC_VH >> 16) + 1  # same trick on bf16 (top 16 bits of f32)


def _register_rsqrt_op():
    """rstd = NR2 rsqrt of Src0, seeded by Src1.  With Src0 = (var+eps)/2
    this converges to 1/sqrt(var+eps) (the halved input IS the 0.5 factor
    of the Newton iteration)."""
    import concourse.dve_ops as dve_ops
    from concourse.dve_ops import DveOp
    from concourse.dve_spec import C0, Spec, Src0, Src1, lower, _has_src1
    from concourse.dve_uop import DveOpSpec

    name = "RSQRT_NR2_ANT"
    if name in dve_ops._SUB_OPCODE_FOR_NAME:
        for op in dve_ops.OPS:
            if op.name == name:
                return op

    def _ref(in0, in1, c0, c1, c2):
        vh = in0.astype(np.float32)
        s = in1.astype(np.float32)
        y1 = s * (c0 - vh * s * s)
        y2 = y1 * (c0 - vh * y1 * y1)
        return y2.astype(np.float32)

    _y1 = Src1 * (C0 - Src0 * (Src1 * Src1))
    spec = Spec(body=_y1 * (C0 - Src0 * (_y1 * _y1)), reference=_ref)
    opcode = dve_ops._CUSTOM_DVE_ROW_BASE + len(dve_ops.OPS)
    assert opcode < 0x20
    dve_ops._SUB_OPCODE_FOR_NAME[name] = opcode
    shas = {}
    for ver in ("v3", "v4"):
        try:
            uops = lower(spec, ver=ver)
            shas[ver] = DveOpSpec(
                name=name, opcode=opcode, uops=uops, rd1_en=_has_src1(spec)
            ).sha(ver)
        except Exception:
            pass
    op = DveOp(name, spec, subdim=False, uops_sha=shas)
    dve_ops.OPS.append(op)
    dve_ops.CUSTOM_DVE_SPECS[name] = spec
    return op


RSQRT_NR2 = _register_rsqrt_op()


# ------------------------------------------------- host-side weight folding
def _prepare_weights(p):
    f = lambda a: np.asarray(a, np.float64)
    C = np.eye(D) - 1.0 / D

    pos = np.arange(10, dtype=np.float64)[:, None]
    div = np.exp(np.arange(0, D, 2, dtype=np.float64) * (-math.log(10000.0) / D))
    pe = np.zeros((10, D))
    pe[:, 0::2] = np.sin(pos * div)
    pe[:, 1::2] = np.cos(pos * div)

    out = {}
    # ---- embedding: two 3-row lhsT (row 2 = bias via ones input row)
    W_in = f(p["w_in"]) * math.sqrt(D)          # [D,2]
    b_in = f(p["b_in"]) * math.sqrt(D)          # [D]
    A_v = C @ W_in
    Cg = C @ np.diag(f(p["g_in"])) @ C
    A_y = Cg @ W_in
    emb = np.zeros((3, 2 * D), np.float32)
    emb[0:2, 0:D] = A_v.T
    emb[2, 0:D] = C @ b_in
    emb[0:2, D:2 * D] = A_y.T
    emb[2, D:2 * D] = Cg @ b_in
    out["emb"] = emb.astype(np.float16)         # [3, 256] fp16 lhsT
    ebias = np.zeros((D, 2), np.float32)
    for t in range(S):
        ebias[:, t] = C @ (f(p["bt_in"]) + pe[t])
    out["ebias"] = ebias

    # ---- per-layer folded weights [128, 1152] fp16 lhsT blocks:
    #  0:128 WqT | 128:256 WkT | 256:384 WvT | 384:512 A1T | 512:640 A2T |
    #  640:896 F1T | 896:1024 F2C_0 | 1024:1152 F2C_1
    wl = np.zeros((L, D, 1152), np.float64)
    blb = np.zeros((L, D, 8), np.float32)
    brow = np.zeros((L, D), np.float64)         # q bias rows (lhsT [1,128])
    for l in range(L):
        g1 = f(p["n1_g"][l]); b1 = f(p["n1_b"][l])
        qkv_w = f(p["qkv_w"][l]); qkv_b = f(p["qkv_b"][l])
        Wq = qkv_w[0:128] * g1[None, :]
        Wk = qkv_w[128:256] * g1[None, :]
        Wv = qkv_w[256:384] * g1[None, :]
        bq = qkv_b[0:128] + qkv_w[0:128] @ b1
        b_v = qkv_b[256:384] + qkv_w[256:384] @ b1
        out_w = f(p["out_w"][l]); out_b = f(p["out_b"][l])
        A1 = 0.5 * C @ out_w @ Wv
        A2 = 0.5 * C @ out_w
        b_h2 = C @ (out_b + out_w @ b_v)
        g2 = f(p["n2_g"][l]); b2 = f(p["n2_b"][l])
        ff1_w = f(p["ff1_w"][l]); ff1_b = f(p["ff1_b"][l])
        F1 = ff1_w * g2[None, :]
        bff1 = ff1_b + ff1_w @ b2
        F2C = C @ f(p["ff2_w"][l])
        b_h3 = C @ f(p["ff2_b"][l])
        wl[l, :, 0:128] = Wq.T
        wl[l, :, 128:256] = Wk.T
        wl[l, :, 256:384] = Wv.T
        wl[l, :, 384:512] = A1.T
        wl[l, :, 512:640] = A2.T
        wl[l, :, 640:896] = F1.T
        wl[l, :, 896:1024] = F2C.T[0:128]
        wl[l, :, 1024:1152] = F2C.T[128:256]
        blb[l, :, 1] = b_h2
        blb[l, :, 2] = bff1[0:128]
        blb[l, :, 3] = bff1[128:256]
        blb[l, :, 4] = b_h3
        blb[l, :, 5] = b_h2 / 16.0
        blb[l, :, 6] = b_h3 / 16.0
        brow[l] = bq
    out["wl"] = wl.astype(np.float16)
    out["bl"] = blb
    out["brow"] = brow.astype(np.float16).reshape(L, 1, D)

    # ---- head
    go = f(p["g_out"]); bo = f(p["bt_out"])
    h1_w = f(p["h1_w"])
    wh = np.zeros((D, 193), np.float64)
    wh[:, 0:128] = (0.5 * h1_w * go[None, :]).T
    wh[:, 128:192] = f(p["h2_w"]).T
    wh[0:64, 192] = f(p["h3_w"])[0]
    out["wh"] = wh.astype(np.float16)
    bh = np.zeros((D, 3), np.float32)
    bh[:, 0] = f(p["h1_b"]) + h1_w @ bo
    bh[0:64, 1] = f(p["h2_b"])
    bh[0, 2] = f(p["h3_b"])[0]
    out["bh"] = bh
    return out


def _static_consts():
    c = {}
    c["Jq"] = np.full((128, 128), 1.0 / 256.0, BF)      # var/2 reduce+bcast
    c["I"] = np.eye(128, dtype=np.float16)
    sm = np.zeros((128, 4), np.float16)
    for d in range(128):
        sm[d, d // HD] = 1.0 / math.sqrt(HD)
    c["smask"] = sm
    bc = np.zeros((36, 256), np.float16)
    for d in range(128):
        bc[0 + d // HD, 0 * 128 + d] = 1.0
        bc[32 + d // HD, 1 * 128 + d] = 1.0
    c["bcmask"] = bc
    return c


def _mm(nc, out_ps, lhsT, rhs, start, stop):
    n = rhs.shape[-1]
    nch = (n + MMC - 1) // MMC
    for ci in range(nch):
        sl = slice(ci * MMC, min((ci + 1) * MMC, n))
        nc.tensor.matmul(out_ps[:, sl], lhsT, rhs[:, sl],
                         start=start, stop=stop)


def build_nc(ntiles=None, pair=1, width=8, n=256):
    global N, NT, NTILES
    N, NT, NTILES = n, 2 * n, BP // n
    if ntiles is None:
        ntiles = NTILES
    g = pair * width
    nc = bacc.Bacc(None, target_bir_lowering=False)
    cst = _static_consts()

    x_d = nc.dram_tensor("x", [BP, 4], F32, kind="ExternalInput")
    wl_d = nc.dram_tensor("wl", [L, 128, 1152], F16, kind="ExternalInput")
    bl_d = nc.dram_tensor("bl", [L, 128, 8], F32, kind="ExternalInput")
    brow_d = nc.dram_tensor("brow", [L, 1, 128], F16, kind="ExternalInput")
    emb_d = nc.dram_tensor("emb", [3, 256], F16, kind="ExternalInput")
    ebias_d = nc.dram_tensor("ebias", [128, 2], F32, kind="ExternalInput")
    wh_d = nc.dram_tensor("wh", [128, 193], F16, kind="ExternalInput")
    bh_d = nc.dram_tensor("bh", [128, 3], F32, kind="ExternalInput")
    o_d = nc.dram_tensor("o", [1, BP], F32, kind="ExternalOutput")

    I_d = nc.inline_tensor(cst["I"], name="Imat")
    sm_d = nc.inline_tensor(cst["smask"], name="smask")
    bc_d = nc.inline_tensor(cst["bcmask"], name="bcmask")

    with tile.TileContext(nc) as tc, ExitStack() as ctx:
        wp = ctx.enter_context(tc.tile_pool(name="weights", bufs=1))
        hp = ctx.enter_context(
            tc.tile_pool(name="hbuf", bufs=g + 2))
        sp = ctx.enter_context(
            tc.tile_pool(name="scratch", bufs=min(g + 1, 6)))
        nbufs = 6 if NT <= 512 else 3
        psB = ctx.enter_context(
            tc.tile_pool(name="psB", bufs=nbufs, space="PSUM"))
        psS = ctx.enter_context(tc.tile_pool(name="psS", bufs=2, space="PSUM"))

        def wtile(src, shape, dt, tag):
            t = wp.tile(shape, dt, tag=tag, name=tag)
            nc.sync.dma_start(t[:], src)
            return t

        wl_t = [wtile(wl_d[l], [128, 1152], F16, f"wl{l}") for l in range(L)]
        bl_t = [wtile(bl_d[l], [128, 8], F32, f"bl{l}") for l in range(L)]
        brow_t = [wtile(brow_d[l], [1, 128], F16, f"brow{l}") for l in range(L)]
        bh_t = wtile(bh_d[:], [128, 3], F32, "bh")
        wh_t = wtile(wh_d[:], [128, 193], F16, "wh")
        eb_t = wtile(ebias_d[:], [128, 2], F32, "eb")
        I_t = wtile(I_d[:], [128, 128], F16, "Im")
        sm_t = wtile(sm_d[:], [128, 4], F16, "sm")
        bc_t = wtile(bc_d[:], [36, 256], F16, "bc")
        emb_t = wtile(emb_d[:], [3, 256], F16, "embr")
        ones_s = sp.tile([1, NT], F16, tag="ones_s", name="ones_s")
        nc.vector.memset(ones_s[:], 1.0)
        ones_t = wp.tile([1, NT], F16, tag="ones", name="ones")
        nc.vector.tensor_copy(ones_t[:], ones_s[:])

        def bcast2(ps_ap):
            """[128,N] AP -> [128,2,N] with free-dim broadcast."""
            return bass.AP(tensor=ps_ap.tensor, offset=ps_ap.offset,
                           ap=[ps_ap.ap[0], [0, 2], ps_ap.ap[1]])

        def braid_ln(hcs, tag, sq_srcs=None, alt=False):
            """Braided LN over a group: y_i = hc_i * rsqrt(var_i + eps).
            sq = (x/16)^2 via the Act scale slot, var/2 by gpsimd
            partition_all_reduce, eps via Pool TS, magic-seed + NR2 on DVE.
            sq_srcs[i] = (psum_ap, bias16_ap) squares straight from PSUM."""
            sqs = []
            for i, hc in enumerate(hcs):
                sq = sp.tile([128, NT], BF16, tag="sqln", name=f"sq{tag}{i}")
                if sq_srcs is not None:
                    ps, bias16 = sq_srcs[i]
                    nc.scalar.activation(out=sq[:], in_=ps, func=AF.Square,
                                         bias=bias16, scale=1.0 / 16.0)
                else:
                    nc.scalar.activation(out=sq[:], in_=hc[:], func=AF.Square,
                                         bias=0.0, scale=1.0 / 16.0)
                sqs.append(sq)
            vhs = []
            for i, sq in enumerate(sqs):
                vh = sp.tile([128, NT], BF16, tag="vhln", name=f"vh{tag}{i}")
                nc.gpsimd.partition_all_reduce(vh[:], sq[:], 128,
                                               ReduceOp.add)
                vhs.append(vh)
            for vh in vhs:
                nc.gpsimd.tensor_scalar(vh[:], vh[:], EPS / 2, None,
                                        op0=ALU.add)
            tbs = []
            for i, vh in enumerate(vhs):
                tb = sp.tile([128, NT], I16, tag="tbln", name=f"tb{tag}{i}")
                nc.vector.tensor_scalar(
                    tb[:], vh[:].bitcast(I16), 1, -1,
                    op0=ALU.logical_shift_right, op1=ALU.bitwise_xor)
                tbs.append(tb)
            for tb in tbs:
                if alt:
                    nc.vector.tensor_scalar(tb[:], tb[:], SEED_ADD16, None,
                                            op0=ALU.add)
                else:
                    nc.gpsimd.tensor_scalar(tb[:], tb[:], SEED_ADD16, None,
                                            op0=ALU.add)
            Rs = []
            for i, (vh, tb) in enumerate(zip(vhs, tbs)):
                R = sp.tile([128, NT], F16, tag="Rln", name=f"R{tag}{i}")
                nc.vector._custom_dve(RSQRT_NR2, out=R[:], in0=vh[:],
                                      in1=tb[:].bitcast(BF16), s0=1.5)
                Rs.append(R)
            ys = []
            for i, (hc, R) in enumerate(zip(hcs, Rs)):
                y = sp.tile([128, NT], F16, tag="yln", name=f"y{tag}{i}")
                if alt:
                    nc.vector.tensor_mul(y[:], hc[:], R[:])
                else:
                    nc.gpsimd.tensor_tensor(y[:], hc[:], R[:], op=ALU.mult)
                ys.append(y)
            return ys

        def braid_embed(its, st):
            xts = []
            for i, it in enumerate(its):
                b0 = it * N
                xt = sp.tile([3, NT], F32, tag="xt", name=f"xt{i}")
                nc.vector.memset(xt[0:3, :], 1.0)
                xs = x_d[b0:b0 + N, :]
                nc.sync.dma_start(xt[0:2, 0:N],
                                  xs.rearrange("n f -> f n")[0:2, :])
                nc.sync.dma_start(xt[0:2, N:NT],
                                  xs.rearrange("n f -> f n")[2:4, :])
                xtc = sp.tile([3, NT], F16, tag="xtc", name=f"xtc{i}")
                nc.scalar.copy(xtc[:], xt[:])
                xts.append(xtc)
            vcs, ygs = [], []
            for i, xt in enumerate(xts):
                r = xt[:]
                vc = psB.tile([128, NT], F32, tag="ps", name=f"vc{i}")
                nc.tensor.matmul(vc[:, 0:N], emb_t[:, 0:128], r[:, 0:N],
                                 start=True, stop=True)
                nc.tensor.matmul(vc[:, N:NT], emb_t[:, 0:128], r[:, N:NT],
                                 start=True, stop=True)
                vcs.append(vc)
            for i, xt in enumerate(xts):
                r = xt[:]
                yg = psB.tile([128, NT], F32, tag="ps", name=f"yg{i}")
                nc.tensor.matmul(yg[:, 0:N], emb_t[:, 128:256], r[:, 0:N],
                                 start=True, stop=True)
                nc.tensor.matmul(yg[:, N:NT], emb_t[:, 128:256], r[:, N:NT],
                                 start=True, stop=True)
                ygs.append(yg)
            yield
            sqs = []
            for i, vc in enumerate(vcs):
                sq = sp.tile([128, NT], BF16, tag="sqln", name=f"sqe{i}")
                nc.scalar.activation(out=sq[:], in_=vc[:], func=AF.Square,
                                     bias=0.0, scale=1.0 / 16.0)
                sqs.append(sq)
            vhs = []
            for i, sq in enumerate(sqs):
                vh = sp.tile([128, NT], BF16, tag="vhln", name=f"vhe{i}")
                nc.gpsimd.partition_all_reduce(vh[:], sq[:], 128,
                                               ReduceOp.add)
                vhs.append(vh)
            for vh in vhs:
                nc.gpsimd.tensor_scalar(vh[:], vh[:], EPS / 2, None,
                                        op0=ALU.add)
            tbs = []
            for i, vh in enumerate(vhs):
                tb = sp.tile([128, NT], I16, tag="tbln", name=f"tbe{i}")
                nc.vector.tensor_scalar(
                    tb[:], vh[:].bitcast(I16), 1, -1,
                    op0=ALU.logical_shift_right, op1=ALU.bitwise_xor)
                tbs.append(tb)
            for tb in tbs:
                nc.vector.tensor_scalar(tb[:], tb[:], SEED_ADD16, None,
                                        op0=ALU.add)
            Rs = []
            for i, (vh, tb) in enumerate(zip(vhs, tbs)):
                R = sp.tile([128, NT], F16, tag="Rln", name=f"Re{i}")
                nc.vector._custom_dve(RSQRT_NR2, out=R[:], in0=vh[:],
                                      in1=tb[:].bitcast(BF16), s0=1.5)
                Rs.append(R)
            hcs = []
            for i, (yg, R) in enumerate(zip(ygs, Rs)):
                hc = hp.tile([128, NT], F16, tag="h", name=f"hemb{i}")
                nc.vector.tensor_mul(hc[:], yg[:], R[:])
                hcs.append(hc)
            for hc in hcs:
                nc.vector.tensor_scalar(hc[:, 0:N], hc[:, 0:N], eb_t[:, 0:1],
                                        None, op0=ALU.add)
                nc.vector.tensor_scalar(hc[:, N:NT], hc[:, N:NT],
                                        eb_t[:, 1:2], None, op0=ALU.add)
            st["hcs"] = hcs

        def braid_layer(l, st):
            hcs = st["hcs"]
            prev_ps = st["pps"]
            W = wl_t[l]
            Bb = bl_t[l]
            ys1 = braid_ln(hcs, f"1_{l}", sq_srcs=prev_ps, alt=True)
            yield
            yds, yss = [], []
            for i, y1 in enumerate(ys1):
                yd = sp.tile([128, N], F16, tag="yd", name=f"yd{i}")
                nc.vector.tensor_tensor(yd[:], y1[:, 0:N], y1[:, N:NT],
                                        op=ALU.subtract)
                yds.append(yd)
            for i, y1 in enumerate(ys1):
                ysum = sp.tile([128, N], F16, tag="ys", name=f"ys{i}")
                nc.vector.tensor_tensor(ysum[:], y1[:, 0:N], y1[:, N:NT],
                                        op=ALU.add)
                yss.append(ysum)
            qs = []
            for i, y1 in enumerate(ys1):
                q = psB.tile([128, NT], F32, tag="ps", name=f"q{i}")
                nc.tensor.matmul(q[:], W[:, 0:128], y1[:],
                                 start=True, stop=False)
                nc.tensor.matmul(q[:], brow_t[l][:], ones_t[:],
                                 start=False, stop=True)
                qs.append(q)
            dkvs = []
            for i, yd in enumerate(yds):
                dkv = psB.tile([128, NT], F32, tag="ps", name=f"dkv{i}")
                nc.tensor.matmul(dkv[:, 0:N], W[:, 128:256], yd[:],
                                 start=True, stop=True)
                nc.tensor.matmul(dkv[:, N:NT], W[:, 256:384], yd[:],
                                 start=True, stop=True)
                dkvs.append(dkv)
            dkvs_sb = []
            for i, dkv in enumerate(dkvs):
                dsb = sp.tile([128, NT], F16, tag="dkvsb", name=f"dkvsb{i}")
                nc.scalar.activation(out=dsb[:], in_=dkv[:], func=AF.Identity,
                                     bias=0.0, scale=1.0)
                dkvs_sb.append(dsb)
            prs = []
            for i, (q, dsb) in enumerate(zip(qs, dkvs_sb)):
                pr = sp.tile([128, 2, N], F16, tag="pr", name=f"pr{i}")
                nc.vector.tensor_mul(
                    pr[:], q[:].rearrange("p (q n) -> p q n", q=2),
                    bcast2(dsb[:, 0:N]))
                prs.append(pr)
            tbps = []
            dps = []
            for i, pr in enumerate(prs):
                tbp = psB.tile([128, NT], F32, tag="ps", name=f"tbq{i}")
                d = tbp[0:36, 0:N]
                nc.tensor.matmul(tbp[0:4, 0:N], sm_t[:], pr[:, 0, :],
                                 start=True, stop=True)
                nc.tensor.matmul(tbp[32:36, 0:N], sm_t[:], pr[:, 1, :],
                                 start=True, stop=True, tile_position=(0, 32))
                tbps.append(tbp)
                dps.append(d)
            T8s = []
            for i, d in enumerate(dps):
                T8 = sp.tile([36, N], F16, tag="T8", name=f"T8{i}")
                nc.scalar.activation(out=T8[:], in_=d[:], func=AF.Tanh,
                                     bias=0.0, scale=0.5)
                T8s.append(T8)
            for i, (tbp, T8) in enumerate(zip(tbps, T8s)):
                nc.tensor.matmul(tbp[:, 0:N], bc_t[:, 0:128], T8[:],
                                 start=True, stop=True)
                nc.tensor.matmul(tbp[:, N:NT], bc_t[:, 128:256], T8[:],
                                 start=True, stop=True)
            us = []
            for i, (tbp, dsb) in enumerate(zip(tbps, dkvs_sb)):
                u = sp.tile([128, NT], F16, tag="u", name=f"u{i}")
                nc.vector.tensor_mul(
                    u[:].rearrange("p (q n) -> p q n", q=2),
                    tbp[:].rearrange("p (q n) -> p q n", q=2),
                    bcast2(dsb[:, N:NT]))
                us.append(u)
            p1s = []
            for i, (hc, ysum, u) in enumerate(zip(hcs, yss, us)):
                p1 = psB.tile([128, NT], F32, tag="ps", name=f"p1{i}")
                nc.tensor.matmul(p1[:], I_t[:], hc[:],
                                 start=True, stop=False)
                for ci in range(2):
                    sl = slice(ci * N, (ci + 1) * N)
                    nc.tensor.matmul(p1[:, sl], W[:, 384:512], ysum[:],
                                     start=False, stop=False)
                    nc.tensor.matmul(p1[:, sl], W[:, 512:640], u[:, sl],
                                     start=False, stop=True)
                p1s.append(p1)
            hc2s = []
            for i, p1 in enumerate(p1s):
                hc2 = hp.tile([128, NT], F16, tag="h", name=f"h2_{l}_{i}")
                nc.scalar.activation(out=hc2[:], in_=p1[:], func=AF.Identity,
                                     bias=Bb[:, 1:2], scale=1.0)
                hc2s.append(hc2)
            yield
            ys2 = braid_ln(hc2s, f"2_{l}",
                           sq_srcs=[(p1[:], Bb[:, 5:6]) for p1 in p1s],
                           alt=True)
            yield
            f0s, f1s = [], []
            for i, y2 in enumerate(ys2):
                f0 = psB.tile([128, NT], F32, tag="ps", name=f"f0{i}")
                nc.tensor.matmul(f0[:], W[:, 640:768], y2[:],
                                 start=True, stop=True)
                f0s.append(f0)
            for i, y2 in enumerate(ys2):
                f1 = psB.tile([128, NT], F32, tag="ps", name=f"f1{i}")
                nc.tensor.matmul(f1[:], W[:, 768:896], y2[:],
                                 start=True, stop=True)
                f1s.append(f1)
            g0s, g1s = [], []
            for i, f0 in enumerate(f0s):
                g0 = sp.tile([128, NT], F16, tag="g0", name=f"g0{i}")
                nc.scalar.activation(out=g0[:], in_=f0[:], func=AF.Gelu,
                                     bias=Bb[:, 2:3], scale=1.0)
                g0s.append(g0)
            for i, f1 in enumerate(f1s):
                g1 = sp.tile([128, NT], F16, tag="g1", name=f"g1{i}")
                nc.scalar.activation(out=g1[:], in_=f1[:], func=AF.Gelu,
                                     bias=Bb[:, 3:4], scale=1.0)
                g1s.append(g1)
            p2s = []
            for i, (hc2, g0, g1) in enumerate(zip(hc2s, g0s, g1s)):
                p2 = psB.tile([128, NT], F32, tag="ps", name=f"p2{i}")
                nc.tensor.matmul(p2[:], I_t[:], hc2[:],
                                 start=True, stop=False)
                nc.tensor.matmul(p2[:], W[:, 896:1024], g0[:],
                                 start=False, stop=False)
                nc.tensor.matmul(p2[:], W[:, 1024:1152], g1[:],
                                 start=False, stop=True)
                p2s.append(p2)
            hc3s = []
            for i, p2 in enumerate(p2s):
                hc3 = hp.tile([128, NT], F16, tag="h", name=f"h3_{l}_{i}")
                nc.scalar.activation(out=hc3[:], in_=p2[:], func=AF.Identity,
                                     bias=Bb[:, 4:5], scale=1.0)
                hc3s.append(hc3)
            st["hcs"] = hc3s
            st["pps"] = [(p2[:], Bb[:, 6:7]) for p2 in p2s]

        def braid_head(its, st):
            yfs = braid_ln(st["hcs"], "f", sq_srcs=st["pps"], alt=True)
            yield
            p3s = []
            for i, yf in enumerate(yfs):
                p3 = psS.tile([128, N], F32, tag="pss2", name=f"p3{i}")
                nc.tensor.matmul(p3[:], wh_t[:, 0:128], yf[:, 0:N],
                                 start=True, stop=False)
                nc.tensor.matmul(p3[:], wh_t[:, 0:128], yf[:, N:NT],
                                 start=False, stop=True)
                p3s.append(p3)
            p1hs = []
            for i, p3 in enumerate(p3s):
                p1h = sp.tile([128, N], F16, tag="p1h", name=f"p1h{i}")
                nc.scalar.activation(out=p1h[:], in_=p3[:], func=AF.Gelu,
                                     bias=bh_t[:, 0:1], scale=1.0)
                p1hs.append(p1h)
            p4s = []
            for i, p1h in enumerate(p1hs):
                p4 = psS.tile([64, N], F32, tag="pss2", name=f"p4{i}")
                nc.tensor.matmul(p4[:], wh_t[:, 128:192], p1h[:],
                                 start=True, stop=True)
                p4s.append(p4)
            p2hs = []
            for i, p4 in enumerate(p4s):
                p2h = sp.tile([64, N], F16, tag="p2h", name=f"p2h{i}")
                nc.scalar.activation(out=p2h[:], in_=p4[:], func=AF.Gelu,
                                     bias=bh_t[0:64, 1:2], scale=1.0)
                p2hs.append(p2h)
            p5s = []
            for i, p2h in enumerate(p2hs):
                p5 = psS.tile([1, N], F32, tag="pss2", name=f"p5{i}")
                nc.tensor.matmul(p5[:], wh_t[0:64, 192:193], p2h[:],
                                 start=True, stop=True)
                p5s.append(p5)
            ths = []
            for i, p5 in enumerate(p5s):
                th = sp.tile([1, N], F32, tag="th", name=f"th{i}")
                nc.scalar.activation(out=th[:], in_=p5[:], func=AF.Tanh,
                                     bias=bh_t[0:1, 2:3], scale=1.0)
                ths.append(th)
            for i, (it, th) in enumerate(zip(its, ths)):
                b0 = it * N
                res = sp.tile([1, N], F32, tag="res", name=f"res{i}")
                nc.vector.tensor_scalar(res[:], th[:], 3.0, None,
                                        op0=ALU.mult)
                nc.sync.dma_start(o_d[0:1, b0:b0 + N], res[:])

        def gen_pair(its):
            st = {"pps": None}
            yield from braid_embed(its, st)
            yield
            for l in range(L):
                yield from braid_layer(l, st)
                yield
            yield from braid_head(its, st)

        pending = [list(range(s, min(s + pair, ntiles)))
                   for s in range(0, ntiles, pair)]
        active = []
        while pending or active:
            if pending and len(active) < width:
                active.append(gen_pair(pending.pop(0)))
            for gen in list(active):
                try:
                    next(gen)
                except StopIteration:
                    active.remove(gen)

    nc.compile()
    return nc


_NC_CACHE = {}

WEIGHT_KEYS = ["wl", "bl", "brow", "emb", "ebias", "wh", "bh"]


def kernel(**inputs):
    w = _prepare_weights(inputs)
    if "nc" not in _NC_CACHE:
        _NC_CACHE["nc"] = build_nc()
    nc = _NC_CACHE["nc"]
    x = np.asarray(inputs["x"], np.float32)
    in_maps = []
    for c in range(NCORES):
        m = {"x": np.ascontiguousarray(x[c * BP:(c + 1) * BP])}
        for k in WEIGHT_KEYS:
            m[k] = w[k]
        in_maps.append(m)
    res = run_bass_kernel_spmd(nc, in_maps, core_ids=list(range(NCORES)))
    outs = [res.results[c]["o"].reshape(BP, 1) for c in range(NCORES)]
    return np.concatenate(outs, axis=0).astype(np.float32)


if __name__ == "__main__":
    build_nc(ntiles=2, width=2)
    print("build ok")
